# revision 15
# baseline (speedup 1.0000x reference)
"""Biaffine edge attention on 8 Trainium2 NeuronCores.

out[b,i,j] = head[b,i,:] @ edge_U @ dep[b,j,:] + head[b,i,:]@w1 + dep[b,j,:]@w2 + b0

Sharding: data-parallel over batch (B=8, one batch per core).

Layout strategy: head/dep are transposed on the host (pure relayout, like the
baseline's U relayout) so the device does ZERO PE transposes:
  HT[d,i] = head[b,i,d],  PT[k,j] = dep[b,j,k]   (bf16, per-128-row blocks)
  mm1: T1T[k,i] = sum_d U[d,k] HT[d,i]   lhsT = U row-block (natural layout)
  mm2: out[i,j] = sum_k T1T[k,i] PT[k,j] lhsT = T1T (mm1's natural output)

All matmul operands are bf16: same PE rate as fp32r (1 cycle/row) but half
the DMA traffic and SBUF footprint. PSUM accumulates fp32; rel err ~4e-3.

Every 128-row block lives in its OWN SBUF tile so DMA->matmul dependencies
are exact (slices of one big tile made mm1 wait on unrelated later DMAs).
mm1 runs dt-outer across all 8 PSUM banks so round dt needs only u[dt] +
ht[dt]. A DMA'd bf16 identity feeds warmup matmuls that keep the PE busy
from the end of the ~7us framework preamble until the first data lands --
any PE idle gap resets the HAM clock ramp (1.2 GHz for ~6us after a gap).

s_head/s_dep are [1,S] row matmuls (all four back-to-back on PE), s_head is
flipped to a per-partition column with 8 tiny transposes, bias folds into
s_head. Epilogue: DVE scalar_tensor_tensor -> bf16 out tile -> DMA (host
casts back to fp32). The last mm2 group is column-split so the tail chain
(matmul -> STT -> out DMA) is half length.
"""

import numpy as np
import ml_dtypes

import concourse.bass as bass
import concourse.mybir as mybir
import concourse.tile as tile
from concourse import bacc
from concourse.bass_utils import run_bass_kernel_spmd

B, S, D = 8, 1024, 1024
P = 128
DO = D // P  # 8
NH = 512     # fp32 PSUM bank free size
NWARM = 40
F32 = mybir.dt.float32
BF16 = mybir.dt.bfloat16
ADD = mybir.AluOpType.add

_CACHE = {}


def build_nc(nwarm=NWARM):
    nc = bacc.Bacc(None, target_bir_lowering=False)

    # host-pretransposed inputs, all bf16
    ht = nc.dram_tensor("ht", [DO, P, S], BF16, kind="ExternalInput")   # [dt, dd, i]
    pt = nc.dram_tensor("pt", [DO, P, S], BF16, kind="ExternalInput")   # [kt, kk, j]
    u = nc.dram_tensor("u", [DO, P, D], BF16, kind="ExternalInput")     # [dt, dd, k]
    wc = nc.dram_tensor("wc", [P, 2 * DO], BF16, kind="ExternalInput")  # w1|w2 cols
    bias0 = nc.dram_tensor("bias0", [1, 1], F32, kind="ExternalInput")
    out = nc.dram_tensor("out", [S, S], BF16, kind="ExternalOutput")

    with tile.TileContext(nc) as tc:
        with (
            tc.tile_pool(name="const", bufs=1) as const,
            tc.tile_pool(name="big", bufs=1) as big,
            tc.tile_pool(name="outp", bufs=4) as outp,
            tc.tile_pool(name="ps", bufs=8, space="PSUM") as psp,
        ):
            # warmup operand: all-ones via gpsimd memset (no DMA dep, ready
            # before the Tensor queue preamble ends) so the PE starts
            # immediately, which opens the HAM clock-ramp window as early as
            # possible.
            warm_src = const.tile([P, P], BF16)
            nc.gpsimd.memset(warm_src[:], 1.0)
            one_sb = const.tile([1, 1], F32)
            nc.gpsimd.memset(one_sb[:], 1.0)

            wc_sb = const.tile([P, 2 * DO], BF16)
            b_sb = const.tile([1, 1], F32)
            shead_col = const.tile([P, DO], F32)
            row_sb = const.tile([1, S], F32)     # s_head + bias
            drow_sb = const.tile([1, S], F32)    # s_dep
            sdep_full = const.tile([P, S], F32)

            u_t = [big.tile([P, D], BF16, tag=f"u{i}", name=f"u{i}")
                   for i in range(DO)]
            ht_t = [big.tile([P, S], BF16, tag=f"ht{i}", name=f"ht{i}")
                    for i in range(DO)]
            pt_t = [big.tile([P, S], BF16, tag=f"pt{i}", name=f"pt{i}")
                    for i in range(DO)]
            t1t_t = [big.tile([P, S], BF16, tag=f"t1t{i}", name=f"t1t{i}")
                     for i in range(DO)]

            # ---------- DMA emission (sync ring is FIFO: order = priority) --
            # mm1-ih0 needs u[dt] + left half of ht[dt]: stream those first,
            # then the right ht halves (for ih1). pt / w / bias go on the
            # gpsimd ring so the sync ring's completion pipe stays short.
            for dt in range(DO):
                nc.sync.dma_start(u_t[dt][:], u[dt])
                nc.sync.dma_start(ht_t[dt][:, 0:NH], ht[dt][:, 0:NH])
            for dt in range(DO):
                nc.sync.dma_start(ht_t[dt][:, NH:S], ht[dt][:, NH:S])
            for kt in range(DO):
                nc.gpsimd.dma_start(pt_t[kt][:], pt[kt])
            nc.gpsimd.dma_start(wc_sb[:], wc[:])
            nc.gpsimd.dma_start(b_sb[:], bias0[:])

            # ---------- PE warmup: real matmuls inside the DMA shadow -------
            warm_ps = psp.tile([P, NH], F32, tag="ps")
            for _ in range(nwarm):
                nc.tensor.matmul(
                    warm_ps[:, 0:P], warm_src[:], warm_src[:], start=True, stop=True
                )

            copy_i = [0]

            def copy(dst, src):
                if copy_i[0] % 2 == 0:
                    nc.scalar.copy(dst, src)
                else:
                    nc.vector.tensor_copy(dst, src)
                copy_i[0] += 1

            # ---------- mm1 (dt-outer over all 8 PSUM banks) ----------------
            for ih in range(2):
                ps1 = [
                    psp.tile([P, NH], F32, tag="ps", name=f"ps1_{ih}_{k}")
                    for k in range(DO)
                ]
                for dt in range(DO):
                    for kt in range(DO):
                        nc.tensor.matmul(
                            ps1[kt][:],
                            u_t[dt][:, kt * P:(kt + 1) * P],
                            ht_t[dt][:, ih * NH:(ih + 1) * NH],
                            start=(dt == 0),
                            stop=(dt == DO - 1),
                        )
                for kt in range(DO):
                    copy(t1t_t[kt][:, ih * NH:(ih + 1) * NH], ps1[kt][:])

            # ---------- s_head / s_dep rows: all PE matmuls back-to-back ----
            ps_r = []
            for ih in range(2):
                ps_ri = psp.tile([P, NH], F32, tag="ps", name=f"ps_r{ih}")
                for dt in range(DO):
                    nc.tensor.matmul(
                        ps_ri[0:1, :],
                        wc_sb[:, dt:dt + 1],
                        ht_t[dt][:, ih * NH:(ih + 1) * NH],
                        start=(dt == 0),
                        stop=(dt == DO - 1),
                    )
                nc.vector.tensor_scalar(
                    row_sb[0:1, ih * NH:(ih + 1) * NH],
                    ps_ri[0:1, :], b_sb[0:1, 0:1], None, ADD,
                )
                ps_r.append(ps_ri)
            for jh in range(2):
                ps_d = psp.tile([P, NH], F32, tag="ps", name=f"ps_d{jh}")
                for kt in range(DO):
                    nc.tensor.matmul(
                        ps_d[0:1, :],
                        wc_sb[:, DO + kt:DO + kt + 1],
                        pt_t[kt][:, jh * NH:(jh + 1) * NH],
                        start=(kt == 0),
                        stop=(kt == DO - 1),
                    )
                nc.vector.tensor_copy(
                    drow_sb[0:1, jh * NH:(jh + 1) * NH], ps_d[0:1, :]
                )
                nc.gpsimd.partition_broadcast(
                    sdep_full[:, jh * NH:(jh + 1) * NH],
                    drow_sb[0:1, jh * NH:(jh + 1) * NH],
                )
            # s_head row -> per-partition column (8 tiny PE transposes)
            ps_c = psp.tile([P, NH], F32, tag="ps")
            for it in range(DO):
                nc.tensor.transpose(
                    ps_c[:, it:it + 1],
                    row_sb[0:1, it * P:(it + 1) * P],
                    one_sb[0:1, 0:1],
                )
            nc.scalar.copy(shead_col[:], ps_c[:, 0:DO])

            # ---------- mm2 + epilogue --------------------------------------
            def mm2_group(it, jh, c0, c1):
                ps = psp.tile([P, c1 - c0], F32, tag="ps", name=f"mm2_{it}_{jh}")
                for kt in range(DO):
                    nc.tensor.matmul(
                        ps[:],
                        t1t_t[kt][:, it * P:(it + 1) * P],
                        pt_t[kt][:, jh * NH + c0:jh * NH + c1],
                        start=(kt == 0),
                        stop=(kt == DO - 1),
                    )
                ot = outp.tile([P, c1 - c0], BF16, tag="out", name=f"ot_{it}_{jh}_{c0}")
                nc.vector.scalar_tensor_tensor(
                    out=ot[:], in0=ps[:],
                    scalar=shead_col[:, it:it + 1],
                    in1=sdep_full[:, jh * NH + c0:jh * NH + c1],
                    op0=ADD, op1=ADD,
                )
                nc.sync.dma_start(
                    out[it * P:(it + 1) * P, jh * NH + c0:jh * NH + c1], ot[:]
                )

            for jh in range(2):
                for it in range(DO):
                    if jh == 1 and it == DO - 1:
                        # split the final group so the tail chain is short
                        mm2_group(it, jh, 0, NH // 2)
                        mm2_group(it, jh, NH // 2, NH)
                    else:
                        mm2_group(it, jh, 0, NH)

    nc.compile()
    return nc


def _get_nc(nwarm=NWARM):
    key = ("nc", nwarm)
    if key not in _CACHE:
        _CACHE[key] = build_nc(nwarm)
    return _CACHE[key]


def _in_maps(head, dep, edge_U, edge_W, edge_b):
    bf16 = ml_dtypes.bfloat16
    head = np.asarray(head, dtype=np.float32)
    dep = np.asarray(dep, dtype=np.float32)
    u_prep = np.ascontiguousarray(
        np.asarray(edge_U, dtype=np.float32)
    ).astype(bf16).reshape(DO, P, D)
    w = np.asarray(edge_W, dtype=np.float32).reshape(-1)
    w1c = w[:D].reshape(DO, P).T
    w2c = w[D:].reshape(DO, P).T
    wc = np.ascontiguousarray(np.concatenate([w1c, w2c], axis=1)).astype(bf16)
    b0 = np.asarray(edge_b, dtype=np.float32).reshape(1, 1)
    head_b = head.astype(bf16)
    dep_b = dep.astype(bf16)
    maps = []
    for b in range(B):
        maps.append({
            "ht": np.ascontiguousarray(head_b[b].T).reshape(DO, P, S),
            "pt": np.ascontiguousarray(dep_b[b].T).reshape(DO, P, S),
            "u": u_prep,
            "wc": wc,
            "bias0": b0,
        })
    return maps


def kernel(head, dep, edge_U, edge_W, edge_b, **run_kwargs):
    nc = _get_nc()
    maps = _in_maps(head, dep, edge_U, edge_W, edge_b)
    res = run_bass_kernel_spmd(nc, maps, core_ids=list(range(B)), **run_kwargs)
    out = np.stack(
        [np.asarray(res.results[c]["out"]).astype(np.float32) for c in range(B)],
        axis=0,
    )
    if run_kwargs:
        _CACHE["last_result"] = res
    return out


# revision 17
# speedup vs baseline: 1.0487x; 1.0487x over previous
"""Biaffine edge attention on 8 Trainium2 NeuronCores.

out[b,i,j] = head[b,i,:] @ edge_U @ dep[b,j,:] + head[b,i,:]@w1 + dep[b,j,:]@w2 + b0

Sharding: data-parallel over batch (B=8, one batch per core).

Layout strategy: head/dep are transposed on the host (pure relayout, like the
baseline's U relayout) so the device does ZERO PE transposes:
  HT[d,i] = head[b,i,d],  PT[k,j] = dep[b,j,k]   (bf16, per-128-row blocks)
  mm1: T1T[k,i] = sum_d U[d,k] HT[d,i]   lhsT = U row-block (natural layout)
  mm2: out[i,j] = sum_k T1T[k,i] PT[k,j] lhsT = T1T (mm1's natural output)

All matmul operands are bf16: same PE rate as fp32r (1 cycle/row) but half
the DMA traffic and SBUF footprint. PSUM accumulates fp32; rel err ~4e-3.

Every 128-row block lives in its OWN SBUF tile so DMA->matmul dependencies
are exact (slices of one big tile made mm1 wait on unrelated later DMAs).
mm1 runs dt-outer across all 8 PSUM banks so round dt needs only u[dt] +
ht[dt]. A DMA'd bf16 identity feeds warmup matmuls that keep the PE busy
from the end of the ~7us framework preamble until the first data lands --
any PE idle gap resets the HAM clock ramp (1.2 GHz for ~6us after a gap).

s_head/s_dep are [1,S] row matmuls (all four back-to-back on PE), s_head is
flipped to a per-partition column with 8 tiny transposes, bias folds into
s_head. Epilogue: DVE scalar_tensor_tensor -> bf16 out tile -> DMA (host
casts back to fp32). The last mm2 group is column-split so the tail chain
(matmul -> STT -> out DMA) is half length.
"""

import numpy as np
import ml_dtypes

import concourse.bass as bass
import concourse.mybir as mybir
import concourse.tile as tile
from concourse import bacc
from concourse.bass_utils import run_bass_kernel_spmd

B, S, D = 8, 1024, 1024
P = 128
DO = D // P  # 8
NH = 512     # fp32 PSUM bank free size
NWARM = 30
F32 = mybir.dt.float32
BF16 = mybir.dt.bfloat16
ADD = mybir.AluOpType.add

_CACHE = {}


def build_nc(nwarm=NWARM):
    nc = bacc.Bacc(None, target_bir_lowering=False)

    # host-pretransposed inputs, all bf16
    ht = nc.dram_tensor("ht", [DO, P, S], BF16, kind="ExternalInput")   # [dt, dd, i]
    pt = nc.dram_tensor("pt", [DO, P, S], BF16, kind="ExternalInput")   # [kt, kk, j]
    u = nc.dram_tensor("u", [DO, P, D], BF16, kind="ExternalInput")     # [dt, dd, k]
    wc = nc.dram_tensor("wc", [P, 2 * DO], BF16, kind="ExternalInput")  # w1|w2 cols
    bias0 = nc.dram_tensor("bias0", [1, 1], F32, kind="ExternalInput")
    out = nc.dram_tensor("out", [S, S], BF16, kind="ExternalOutput")

    with tile.TileContext(nc) as tc:
        with (
            tc.tile_pool(name="const", bufs=1) as const,
            tc.tile_pool(name="big", bufs=1) as big,
            tc.tile_pool(name="outp", bufs=4) as outp,
            tc.tile_pool(name="ps", bufs=8, space="PSUM") as psp,
        ):
            # warmup operand: all-ones via gpsimd memset (no DMA dep, ready
            # before the Tensor queue preamble ends) so the PE starts
            # immediately, which opens the HAM clock-ramp window as early as
            # possible.
            warm_src = const.tile([P, P], BF16)
            nc.gpsimd.memset(warm_src[:], 1.0)
            one_sb = const.tile([1, 1], F32)
            nc.gpsimd.memset(one_sb[:], 1.0)

            wc_sb = const.tile([P, 2 * DO], BF16)
            b_sb = const.tile([1, 1], F32)
            shead_col = const.tile([P, DO], F32)
            row_sb = const.tile([1, S], F32)     # s_head + bias
            drow_sb = const.tile([1, S], F32)    # s_dep
            sdep_full = const.tile([P, S], F32)

            u_t = [big.tile([P, D], BF16, tag=f"u{i}", name=f"u{i}")
                   for i in range(DO)]
            ht_t = [big.tile([P, S], BF16, tag=f"ht{i}", name=f"ht{i}")
                    for i in range(DO)]
            pt_t = [big.tile([P, S], BF16, tag=f"pt{i}", name=f"pt{i}")
                    for i in range(DO)]
            t1t_t = [big.tile([P, S], BF16, tag=f"t1t{i}", name=f"t1t{i}")
                     for i in range(DO)]

            # ---------- DMA emission (sync ring is FIFO: order = priority) --
            # mm1-ih0 needs u[dt] + left half of ht[dt]: stream those first,
            # then the right ht halves (for ih1), then pt / w / bias. All on
            # one ring: a second ring would steal HBM bandwidth from the
            # critical u/ht stream (measured).
            for dt in range(DO):
                nc.sync.dma_start(u_t[dt][:], u[dt])
                nc.sync.dma_start(ht_t[dt][:, 0:NH], ht[dt][:, 0:NH])
            for dt in range(DO):
                nc.sync.dma_start(ht_t[dt][:, NH:S], ht[dt][:, NH:S])
            for kt in range(DO):
                nc.sync.dma_start(pt_t[kt][:], pt[kt])
            nc.sync.dma_start(wc_sb[:], wc[:])
            nc.sync.dma_start(b_sb[:], bias0[:])

            # ---------- PE warmup: real matmuls inside the DMA shadow -------
            warm_ps = psp.tile([P, NH], F32, tag="ps")
            for _ in range(nwarm):
                nc.tensor.matmul(
                    warm_ps[:, 0:P], warm_src[:], warm_src[:], start=True, stop=True
                )

            copy_i = [0]

            def copy(dst, src):
                if copy_i[0] % 2 == 0:
                    nc.scalar.copy(dst, src)
                else:
                    nc.vector.tensor_copy(dst, src)
                copy_i[0] += 1

            # ---------- mm1 (dt-outer over all 8 PSUM banks) ----------------
            for ih in range(2):
                ps1 = [
                    psp.tile([P, NH], F32, tag="ps", name=f"ps1_{ih}_{k}")
                    for k in range(DO)
                ]
                for dt in range(DO):
                    for kt in range(DO):
                        nc.tensor.matmul(
                            ps1[kt][:],
                            u_t[dt][:, kt * P:(kt + 1) * P],
                            ht_t[dt][:, ih * NH:(ih + 1) * NH],
                            start=(dt == 0),
                            stop=(dt == DO - 1),
                        )
                for kt in range(DO):
                    copy(t1t_t[kt][:, ih * NH:(ih + 1) * NH], ps1[kt][:])

            # ---------- s_head / s_dep rows: all PE matmuls back-to-back ----
            ps_r = []
            for ih in range(2):
                ps_ri = psp.tile([P, NH], F32, tag="ps", name=f"ps_r{ih}")
                for dt in range(DO):
                    nc.tensor.matmul(
                        ps_ri[0:1, :],
                        wc_sb[:, dt:dt + 1],
                        ht_t[dt][:, ih * NH:(ih + 1) * NH],
                        start=(dt == 0),
                        stop=(dt == DO - 1),
                    )
                nc.vector.tensor_scalar(
                    row_sb[0:1, ih * NH:(ih + 1) * NH],
                    ps_ri[0:1, :], b_sb[0:1, 0:1], None, ADD,
                )
                ps_r.append(ps_ri)
            for jh in range(2):
                ps_d = psp.tile([P, NH], F32, tag="ps", name=f"ps_d{jh}")
                for kt in range(DO):
                    nc.tensor.matmul(
                        ps_d[0:1, :],
                        wc_sb[:, DO + kt:DO + kt + 1],
                        pt_t[kt][:, jh * NH:(jh + 1) * NH],
                        start=(kt == 0),
                        stop=(kt == DO - 1),
                    )
                nc.vector.tensor_copy(
                    drow_sb[0:1, jh * NH:(jh + 1) * NH], ps_d[0:1, :]
                )
                nc.gpsimd.partition_broadcast(
                    sdep_full[:, jh * NH:(jh + 1) * NH],
                    drow_sb[0:1, jh * NH:(jh + 1) * NH],
                )
            # s_head row -> per-partition column (8 tiny PE transposes)
            ps_c = psp.tile([P, NH], F32, tag="ps")
            for it in range(DO):
                nc.tensor.transpose(
                    ps_c[:, it:it + 1],
                    row_sb[0:1, it * P:(it + 1) * P],
                    one_sb[0:1, 0:1],
                )
            nc.scalar.copy(shead_col[:], ps_c[:, 0:DO])

            # ---------- mm2 + epilogue --------------------------------------
            def mm2_group(it, jh, c0, c1):
                ps = psp.tile([P, c1 - c0], F32, tag="ps", name=f"mm2_{it}_{jh}")
                for kt in range(DO):
                    nc.tensor.matmul(
                        ps[:],
                        t1t_t[kt][:, it * P:(it + 1) * P],
                        pt_t[kt][:, jh * NH + c0:jh * NH + c1],
                        start=(kt == 0),
                        stop=(kt == DO - 1),
                    )
                ot = outp.tile([P, c1 - c0], BF16, tag="out", name=f"ot_{it}_{jh}_{c0}")
                nc.vector.scalar_tensor_tensor(
                    out=ot[:], in0=ps[:],
                    scalar=shead_col[:, it:it + 1],
                    in1=sdep_full[:, jh * NH + c0:jh * NH + c1],
                    op0=ADD, op1=ADD,
                )
                nc.sync.dma_start(
                    out[it * P:(it + 1) * P, jh * NH + c0:jh * NH + c1], ot[:]
                )

            for jh in range(2):
                for it in range(DO):
                    if jh == 1 and it == DO - 1:
                        # split the final group so the tail chain is short
                        mm2_group(it, jh, 0, NH // 2)
                        mm2_group(it, jh, NH // 2, NH)
                    else:
                        mm2_group(it, jh, 0, NH)

    nc.compile()
    return nc


def _get_nc(nwarm=NWARM):
    key = ("nc", nwarm)
    if key not in _CACHE:
        _CACHE[key] = build_nc(nwarm)
    return _CACHE[key]


def _in_maps(head, dep, edge_U, edge_W, edge_b):
    bf16 = ml_dtypes.bfloat16
    head = np.asarray(head, dtype=np.float32)
    dep = np.asarray(dep, dtype=np.float32)
    u_prep = np.ascontiguousarray(
        np.asarray(edge_U, dtype=np.float32)
    ).astype(bf16).reshape(DO, P, D)
    w = np.asarray(edge_W, dtype=np.float32).reshape(-1)
    w1c = w[:D].reshape(DO, P).T
    w2c = w[D:].reshape(DO, P).T
    wc = np.ascontiguousarray(np.concatenate([w1c, w2c], axis=1)).astype(bf16)
    b0 = np.asarray(edge_b, dtype=np.float32).reshape(1, 1)
    head_b = head.astype(bf16)
    dep_b = dep.astype(bf16)
    maps = []
    for b in range(B):
        maps.append({
            "ht": np.ascontiguousarray(head_b[b].T).reshape(DO, P, S),
            "pt": np.ascontiguousarray(dep_b[b].T).reshape(DO, P, S),
            "u": u_prep,
            "wc": wc,
            "bias0": b0,
        })
    return maps


def kernel(head, dep, edge_U, edge_W, edge_b, **run_kwargs):
    nc = _get_nc()
    maps = _in_maps(head, dep, edge_U, edge_W, edge_b)
    res = run_bass_kernel_spmd(nc, maps, core_ids=list(range(B)), **run_kwargs)
    out = np.stack(
        [np.asarray(res.results[c]["out"]).astype(np.float32) for c in range(B)],
        axis=0,
    )
    if run_kwargs:
        _CACHE["last_result"] = res
    return out
